# revision 13
# baseline (speedup 1.0000x reference)
"""ArcFace margin loss kernel for 8 TRN2 NeuronCores.

out = S * logits everywhere except at (i, labels[i]) where
out = S * cos(arccos(x) + m) = S*(x*cos(m) - sqrt(1-x^2)*sin(m)).

Sharding: logits [B=256, C=100000] split along C into 8 shards of
[256, 12500] (Partial-FC style), each viewed flat as [128, 25000].

The kernel is HBM-streaming bound, so the shard is moved in bf16
(tolerance is 2e-2; bf16 keeps f32's exponent so the x64 scale of a
bf16 value is exact and the only error is the 2^-9 input quantization).
Each core streams its bf16 shard through SBUF in NT column tiles:
loads on the Sync HWDGE ring, x64 scale on the Vector engine, stores
on the Scalar HWDGE ring.

The margin fixup is precision-critical (cos(arccos(x)+m) amplifies
input error by 1/sqrt(1-x^2) and can land near zero), so it reads the
exact f32 target cosines, packed [1, B] by the host, and computes
y = S*cos(m)*x - S*sin(m)*sqrt(1-x^2) in f32 on one partition
(GpSimd + Scalar ACT sqrt), writing a tiny [1, B] f32 side output that
the host merges into the final array. This keeps the bulk stream free
of any gather/scatter ordering: no post-store indirect DMA tail.
"""

import math

import numpy as np

S = 64.0
MARGIN = 0.5
B, C, M = 256, 100000, 8
CS = C // M            # 12500 classes per core
P = 128                # SBUF partitions
FREE = (B * CS) // P   # 25000 flat bf16 elements per partition
# bulk column tile sizes (flat elements per partition). Two big tiles:
# each DMA descriptor then covers a 25KB contiguous partition line, which
# maximizes the per-DMA-engine byte rate (~25 GB/s vs ~24 at 6KB packets;
# the stream is bound by the slowest of the 16 round-robin engines), and
# the small transfer count minimizes per-transfer semaphore descriptors.
SIZES = [12500] * 2
assert sum(SIZES) == FREE

_graph_cache = {}


def _build_graph():
    import concourse.bacc as bacc
    import concourse.tile as tile
    from concourse import mybir

    bf16 = mybir.dt.bfloat16
    f32 = mybir.dt.float32
    a_c = S * math.cos(MARGIN)
    b_c = S * math.sin(MARGIN)

    nc = bacc.Bacc()
    x = nc.declare_dram_parameter("x", [P, FREE], bf16, isOutput=False)
    tgt = nc.declare_dram_parameter("tgt", [1, B], f32, isOutput=False)
    out = nc.declare_dram_parameter("out", [P, FREE], bf16, isOutput=True)
    fix = nc.declare_dram_parameter("fix", [1, B], f32, isOutput=True)

    with tile.TileContext(nc) as tc:
        with (
            tc.tile_pool(name="bulk", bufs=len(SIZES)) as pool,
            tc.tile_pool(name="fixp", bufs=1) as fp,
        ):
            # ---- bulk x64 scale, streamed in bf16 column tiles.
            # Loads issue from the Sync HWDGE ring, stores from the Scalar
            # (Activation) HWDGE ring, scale on the Vector engine — three
            # independent issue streams, one SBUF slot per tile.
            NT = len(SIZES)
            off = 0
            for k, fsz in enumerate(SIZES):
                sl = slice(off, off + fsz)
                bt = pool.tile([P, fsz], bf16)
                nc.sync.dma_start(bt[:], x[:, sl])
                nc.vector.tensor_scalar_mul(bt[:], bt[:], S)
                nc.scalar.dma_start(out[:, sl], bt[:])
                off += fsz

            # ---- margin fixup on one partition, [1, B] f32, program-ordered
            # after the bulk so its engine ops slot into mid-stream slack.
            # GpSimd ops + the SWDGE queue keep the HWDGE rings clean; sqrt
            # needs the Scalar ACT, two tensor_scalar micro-ops need the DVE
            # (TensorScalarPtr is rejected by NC-v3's Pool engine).
            xt = fp.tile([1, B], f32)
            nc.gpsimd.dma_start(xt[:], tgt[:])
            sq = fp.tile([1, B], f32)
            nc.gpsimd.tensor_mul(sq[:], xt[:], xt[:])
            r = fp.tile([1, B], f32)
            nc.scalar.activation(
                r[:], sq[:], mybir.ActivationFunctionType.Sqrt,
                bias=1.0, scale=-1.0,
            )
            ya = fp.tile([1, B], f32)
            nc.vector.tensor_scalar_mul(ya[:], xt[:], a_c)
            y = fp.tile([1, B], f32)
            nc.vector.scalar_tensor_tensor(
                y[:], r[:], -b_c, ya[:],
                op0=mybir.AluOpType.mult, op1=mybir.AluOpType.add,
            )
            nc.gpsimd.dma_start(fix[:], y[:])
    nc.finalize()
    return nc


def _get_graph():
    if "nc" not in _graph_cache:
        _graph_cache["nc"] = _build_graph()
    return _graph_cache["nc"]


def _make_in_maps(logits, labels):
    import ml_dtypes

    logits = np.asarray(logits, dtype=np.float32)
    labels = np.asarray(labels).astype(np.int64)
    valid = labels != -1
    safe = np.where(valid, labels, 0)
    rows = np.arange(B)
    # exact f32 target cosines, one slot per row (dead slots get a value
    # that keeps sqrt(1-x^2) well-defined; the host never reads them back)
    t = np.where(valid, logits[rows, safe], 0.5).astype(np.float32)
    t = np.ascontiguousarray(t.reshape(1, B))

    bf = logits.astype(ml_dtypes.bfloat16)
    in_maps = []
    for m in range(M):
        shard = np.ascontiguousarray(bf[:, m * CS : (m + 1) * CS]).reshape(
            P, FREE
        )
        in_maps.append({"x": shard, "tgt": t})
    return in_maps


def _assemble(results, labels):
    labels = np.asarray(labels).astype(np.int64)
    valid = labels != -1
    out = np.concatenate(
        [
            np.asarray(results[m]["out"]).astype(np.float32).reshape(B, CS)
            for m in range(M)
        ],
        axis=1,
    )
    # every core computes the identical [1, B] fixup; take core 0's and
    # merge it over the bulk-scaled entries at the target positions
    fixv = np.asarray(results[0]["fix"]).reshape(B)
    rows = np.arange(B)
    out[rows[valid], labels[valid]] = fixv[valid]
    return out


def kernel(logits, labels):
    from concourse.bass_utils import run_bass_kernel_spmd

    nc = _get_graph()
    in_maps = _make_in_maps(np.asarray(logits), labels)
    res = run_bass_kernel_spmd(nc, in_maps, core_ids=list(range(M)))
    return _assemble(res.results, labels)
